# revision 78
# baseline (speedup 1.0000x reference)
"""Trainium2 Bass kernel for ADRiverDynamics (gnn_message_passing).

8 independent point clouds (B*L=8), one per NeuronCore (pure data parallel),
plus one tiny AllReduce for global BatchNorm statistics.

Per-core pipeline (cloud of N=3072 points, C=64 channels, K=16 neighbors):
  S1  bf16 3-way-split tensors A36/B36 for the distance matmul (one 36-row
      bf16 matmul per 512-chunk instead of a 4-pass fp32 matmul; x = h+m+l
      with h,m,l bf16 plus a 4th augmented coordinate carrying (1, -sq_j),
      giving fp32-grade d2 accuracy); PE transposes (fxT = [f|xyz]
      channel-major); head convs; gate conv
  S2  pass A per block: negd' = 2 x_i.x_j - sq_j (PSUM, 1536-wide halves,
      Act copy to SBUF), per-row top-16 via segmented Max/MaxIndex (the
      per-row -sq_i shift is order-invariant, fixed up via v16), merge via
      match_replace + gpsimd rank trick; idx staged through DRAM into the
      wrapped gather layout; softmax pass C pipelined one slice behind:
      gather (Pool) -> PE transposes -> fp16 k-minor fnei -> cos/softmax
      (d2 from saved top-k values, no gather) -> fp16 2x weighted tree ->
      dist stats -> reaction conv chunk
  S3  global-batch BN (AllReduce) + relu + conv2, final combine
"""
import functools
import numpy as np

B, L, N, C, K = 2, 4, 3072, 64, 16
NB = N // 128          # 24 point blocks
TAU = 0.15
BN_EPS = 1e-5
NCORES = 8
BT = 4                 # blocks per gather slice
NSL = NB // BT         # gather slices

WEIGHT_NAMES = ["Wf", "bf", "Wd", "bd", "Wu", "bu", "Wg1", "bg1", "Wg2", "bg2",
                "Wgate", "bgate", "Wr1", "br1", "gamma", "beta", "Wr2", "br2",
                "log_dt"]


def _build(debug=False, nocol=False):
    import contextlib
    from concourse import bacc
    import concourse.bass as bass
    import concourse.tile as tile
    import concourse.mybir as mybir
    from concourse import masks

    f32 = mybir.dt.float32
    bf16 = mybir.dt.bfloat16
    f16 = mybir.dt.float16
    u16 = mybir.dt.uint16
    i16 = mybir.dt.int16
    Alu = mybir.AluOpType
    Act = mybir.ActivationFunctionType
    AX = mybir.AxisListType
    AP = bass.AP

    nc = bacc.Bacc("TRN2", target_bir_lowering=False, debug=False,
                   num_devices=NCORES)

    f_ext = nc.dram_tensor("f", [N, C], f32, kind="ExternalInput")
    xyz_ext = nc.dram_tensor("xyz", [N, 3], f32, kind="ExternalInput")
    wshapes = {"Wf": [3, C], "bf": [3], "Wd": [1, C], "bd": [1], "Wu": [1, C],
               "bu": [1], "Wg1": [C, 3], "bg1": [C], "Wg2": [C, C], "bg2": [C],
               "Wgate": [C, C], "bgate": [C], "Wr1": [C, C + 5], "br1": [C],
               "gamma": [C], "beta": [C], "Wr2": [C, C], "br2": [C],
               "log_dt": [1]}
    w_ext = {k: nc.dram_tensor(k, shp, f32, kind="ExternalInput")
             for k, shp in wshapes.items()}
    out_ext = nc.dram_tensor("out", [N, C], f32, kind="ExternalOutput")
    dbg_ext = {}
    if debug:
        for k, shp in {"d_idx": [128, NB * K], "d_agg": [128, NB * C],
                       "d_uw": [128, NB * K], "d_v16": [128, NB * K],
                       "d_numv": [128, NB * K], "d_heads": [5, N]}.items():
            dbg_ext[k] = nc.dram_tensor(k, shp, f32, kind="ExternalOutput")

    with tile.TileContext(nc) as tc:
        with contextlib.ExitStack() as ctx:
            cpool = ctx.enter_context(tc.tile_pool(name="consts", bufs=1))
            big = ctx.enter_context(tc.tile_pool(name="big", bufs=1))
            dram = ctx.enter_context(tc.tile_pool(name="dram", bufs=1, space="DRAM"))
            psum = ctx.enter_context(tc.tile_pool(name="psum", bufs=2, space="PSUM"))
            ngps = ctx.enter_context(tc.tile_pool(name="ngps", bufs=1, space="PSUM"))
            small = ctx.enter_context(tc.tile_pool(name="small", bufs=1))
            latex = ctx.enter_context(tc.tile_pool(name="latex", bufs=1))
            gatep = ctx.enter_context(tc.tile_pool(name="gate", bufs=1))
            s1stk = contextlib.ExitStack()
            spl = s1stk.enter_context(tc.tile_pool(name="split", bufs=1))
            ptTp = s1stk.enter_context(tc.tile_pool(name="ptTp", bufs=2, space="PSUM"))

            def ps(p, fr):
                return psum.tile([p, fr], f32, tag="ps", name="pst")

            # ---------------- S1a: split tensors (issued first) ----------
            xyz_sb = big.tile([128, NB, 3], f32)
            nc.sync.dma_start(xyz_sb[:], AP(xyz_ext, 0, [[3, 128], [128 * 3, NB], [1, 3]]))

            pmA = spl.tile([128, NB, 12], f32)
            pmB = spl.tile([128, NB, 12], f32)
            h16 = spl.tile([128, NB, 3], bf16)
            hs16 = spl.tile([128, NB, 1], bf16)

            rt2 = float(np.sqrt(2.0))
            yv = spl.tile([128, NB, 3], f32)
            nc.vector.tensor_scalar(yv[:], xyz_sb[:], rt2, None, Alu.mult)
            x2 = spl.tile([128, NB, 3], f32)
            nc.vector.tensor_tensor(x2[:], xyz_sb[:], xyz_sb[:], Alu.mult)
            sq_p = small.tile([128, NB, 1], f32)
            nc.vector.tensor_reduce(sq_p[:], x2[:], axis=AX.X, op=Alu.add)
            nsq = spl.tile([128, NB, 1], f32)
            nc.vector.tensor_scalar(nsq[:], sq_p[:], -1.0, None, Alu.mult)

            r3 = spl.tile([128, NB, 3], f32)
            r3b = spl.tile([128, NB, 3], f32)
            rs = spl.tile([128, NB, 1], f32)
            rs2 = spl.tile([128, NB, 1], f32)
            # h/m/l: bf16-rounded values stored in fp32 (PE transposes fp32)
            nc.vector.tensor_copy(h16[:], yv[:])
            nc.vector.tensor_copy(pmA[:, :, 0:3], h16[:])
            nc.vector.tensor_tensor(r3[:], yv[:], pmA[:, :, 0:3], Alu.subtract)
            nc.vector.tensor_copy(h16[:], r3[:])
            nc.vector.tensor_copy(pmA[:, :, 4:7], h16[:])
            nc.vector.tensor_tensor(r3b[:], r3[:], pmA[:, :, 4:7], Alu.subtract)
            nc.vector.tensor_copy(h16[:], r3b[:])
            nc.vector.tensor_copy(pmA[:, :, 8:11], h16[:])
            nc.vector.memset(pmA[:, :, 3:4], 1.0)
            nc.vector.memset(pmA[:, :, 7:8], 0.0)
            nc.vector.memset(pmA[:, :, 11:12], 0.0)
            nc.vector.tensor_copy(pmB[:, :, 0:3], pmA[:, :, 0:3])
            nc.vector.tensor_copy(pmB[:, :, 4:7], pmA[:, :, 4:7])
            nc.vector.tensor_copy(pmB[:, :, 8:11], pmA[:, :, 8:11])
            nc.vector.tensor_copy(hs16[:], nsq[:])
            nc.vector.tensor_copy(pmB[:, :, 3:4], hs16[:])
            nc.vector.tensor_tensor(rs[:], nsq[:], pmB[:, :, 3:4], Alu.subtract)
            nc.vector.tensor_copy(hs16[:], rs[:])
            nc.vector.tensor_copy(pmB[:, :, 7:8], hs16[:])
            nc.vector.tensor_tensor(rs2[:], rs[:], pmB[:, :, 7:8], Alu.subtract)
            nc.vector.tensor_copy(hs16[:], rs2[:])
            nc.vector.tensor_copy(pmB[:, :, 11:12], hs16[:])

            ident16 = spl.tile([128, 128], f32)
            masks.make_identity(nc, ident16[:])
            TA = spl.tile([12, N], bf16)
            TB = spl.tile([12, N], bf16)
            for src_pm, dst in ((pmA, TA), (pmB, TB)):
                for j in range(6):
                    ptT = ptTp.tile([12, 512], f32, tag="ptT", name="ptT")
                    for q in range(4):
                        b = 4 * j + q
                        nc.tensor.matmul(ptT[:, 128 * q:128 * (q + 1)],
                                         src_pm[:, b, :], ident16[:, :],
                                         is_transpose=True)
                    nc.vector.tensor_copy(dst[:, 512 * j:512 * (j + 1)], ptT[:])
            # A36 rows: [Ah(4) x3, Am(4) x3, Al(4) x3]; B36: [Bh, Bm, Bl](12) x3
            A36 = big.tile([36, N], bf16)
            B36 = big.tile([36, N], bf16)
            for t in range(3):
                for p in range(3):
                    eng = nc.sync if p % 2 == 0 else nc.scalar
                    eng.dma_start(A36[12 * p + 4 * t:12 * p + 4 * t + 4, :],
                                  TA[4 * p:4 * p + 4, :])
                nc.scalar.dma_start(B36[12 * t:12 * (t + 1), :], TB[0:12, :])

            # ---------------- S1b: weights + transposes + convs ----------
            ident = cpool.tile([128, 128], f32)
            masks.make_identity(nc, ident[:])

            WhT = cpool.tile([C, 5], f32)
            nc.sync.dma_start(WhT[:, 0:1], AP(w_ext["Wd"], 0, [[1, C], [C, 1]]))
            nc.sync.dma_start(WhT[:, 1:2], AP(w_ext["Wu"], 0, [[1, C], [C, 1]]))
            nc.sync.dma_start(WhT[:, 2:5], AP(w_ext["Wf"], 0, [[1, C], [C, 3]]))
            bhead = cpool.tile([5, 1], f32)
            nc.sync.dma_start(bhead[0:1, :], AP(w_ext["bd"], 0, [[1, 1], [1, 1]]))
            nc.sync.dma_start(bhead[1:2, :], AP(w_ext["bu"], 0, [[1, 1], [1, 1]]))
            nc.sync.dma_start(bhead[2:5, :], AP(w_ext["bf"], 0, [[1, 3], [1, 1]]))

            WgateT = cpool.tile([C, C], f32)
            nc.sync.dma_start(WgateT[:], AP(w_ext["Wgate"], 0, [[1, C], [C, C]]))
            Wg1T = cpool.tile([5, C], f32)
            nc.vector.memset(Wg1T[:], 0.0)
            nc.sync.dma_start(Wg1T[2:5, :], AP(w_ext["Wg1"], 0, [[1, 3], [3, C]]))
            Wg2T = cpool.tile([C, C], f32)
            nc.sync.dma_start(Wg2T[:], AP(w_ext["Wg2"], 0, [[1, C], [C, C]]))
            Wr1fT = cpool.tile([C, C], f32)
            nc.sync.dma_start(Wr1fT[:], AP(w_ext["Wr1"], 0, [[1, C], [C + 5, C]]))
            Wr1hT = cpool.tile([5, C], f32)
            nc.sync.dma_start(Wr1hT[0:2, :], AP(w_ext["Wr1"], C + 3, [[1, 2], [C + 5, C]]))
            nc.sync.dma_start(Wr1hT[2:5, :], AP(w_ext["Wr1"], C, [[1, 3], [C + 5, C]]))
            Wr2T = cpool.tile([C, C], f32)
            nc.sync.dma_start(Wr2T[:], AP(w_ext["Wr2"], 0, [[1, C], [C, C]]))

            def vec_col(name):
                t = cpool.tile([C, 1], f32, tag=name, name=name + "_v")
                nc.sync.dma_start(t[:], AP(w_ext[name], 0, [[1, C], [1, 1]]))
                return t
            bgate_v = vec_col("bgate")
            bg1_v = vec_col("bg1")
            bg2_v = vec_col("bg2")
            br2_v = vec_col("br2")
            gamma_v = vec_col("gamma")
            beta_v = vec_col("beta")

            zero128 = cpool.tile([128, 1], f32)
            nc.vector.memset(zero128[:], 0.0)
            ones128 = cpool.tile([128, 1], f32)
            nc.vector.memset(ones128[:], 1.0)
            segb64u = cpool.tile([128, 64], u16)
            nc.gpsimd.iota(segb64u[:], pattern=[[384, 8], [0, 8]],
                           channel_multiplier=0)
            rank16 = cpool.tile([128, 16], i16)
            nc.gpsimd.iota(rank16[:], pattern=[[1, 16]], base=1,
                           channel_multiplier=0)
            dtv = cpool.tile([128, 1], f32)

            f_sb = big.tile([128, NB, C], f32)
            nc.sync.dma_start(f_sb[:], AP(f_ext, 0, [[C, 128], [128 * C, NB], [1, C]]))

            # fxT: rows 0:64 f, 64:67 xyz (fp32, gather source + conv input)
            fxT = big.tile([128, N], f32)
            fT = fxT[0:C, :]
            for j in range(6):
                pt = ps(C, 512)
                for q in range(4):
                    b = 4 * j + q
                    nc.tensor.matmul(pt[:, 128 * q:128 * (q + 1)],
                                     f_sb[:, b:b + 1, :], ident[:, :],
                                     is_transpose=True)
                nc.vector.tensor_copy(fxT[0:C, 512 * j:512 * (j + 1)], pt[:])
            for j in range(6):
                pt = ps(3, 512)
                for q in range(4):
                    b = 4 * j + q
                    nc.tensor.matmul(pt[:, 128 * q:128 * (q + 1)],
                                     xyz_sb[:, b:b + 1, :], ident[:, :],
                                     is_transpose=True)
                nc.vector.tensor_copy(fxT[C:C + 3, 512 * j:512 * (j + 1)], pt[:])

            headsT = big.tile([5, N], f32)
            gateT = gatep.tile([C, N], f32)
            for j in range(6):
                sl = slice(512 * j, 512 * (j + 1))
                ph = ps(5, 512)
                nc.tensor.matmul(ph[:], WhT[:], fT[:, sl], start=True, stop=True)
                nc.scalar.activation(headsT[:, sl], ph[:], Act.Identity,
                                     bias=bhead[:], scale=1.0)
                pg = ps(C, 512)
                nc.tensor.matmul(pg[:], WgateT[:], fT[:, sl], start=True, stop=True)
                nc.scalar.activation(gateT[:, sl], pg[:], Act.Sigmoid,
                                     bias=bgate_v[:], scale=1.0)

            hp = small.tile([128, NB, 5], f32)
            pt5 = ps(128, NB * 5)
            for b in range(NB):
                nc.tensor.matmul(pt5[:, 5 * b:5 * (b + 1)],
                                 headsT[:, 128 * b:128 * (b + 1)], ident[0:5, 0:5],
                                 is_transpose=True)
            nc.vector.tensor_copy(hp[:], pt5[:])

            flow_p = hp[:, :, 2:5]
            # de = softplus(dpre) * (1 + sigmoid(upre))
            de = small.tile([128, NB, 1], f32)
            sgu = small.tile([128, NB, 1], f32)
            nc.scalar.activation(sgu[:], hp[:, :, 1:2], Act.Sigmoid,
                                 bias=zero128[:], scale=1.0)
            nc.vector.tensor_scalar(sgu[:], sgu[:], 1.0, None, Alu.add)
            tmp_b = small.tile([128, NB, 1], f32)
            nc.scalar.activation(tmp_b[:], hp[:, :, 0:1], Act.Exp,
                                 bias=zero128[:], scale=1.0)
            nc.vector.tensor_scalar(tmp_b[:], tmp_b[:], 1.0, None, Alu.add)
            nc.scalar.activation(tmp_b[:], tmp_b[:], Act.Ln,
                                 bias=zero128[:], scale=1.0)
            nc.vector.tensor_tensor(de[:], tmp_b[:], sgu[:], Alu.mult)
            de16 = small.tile([128, NB, 1], f32)
            nc.vector.tensor_scalar(de16[:], de[:], 1.0 / K, None, Alu.mult)

            nc.sync.dma_start(dtv[:], AP(w_ext["log_dt"], 0, [[0, 128], [1, 1]]))
            nc.scalar.activation(dtv[:], dtv[:], Act.Exp, bias=zero128[:], scale=1.0)
            nc.vector.tensor_scalar(dtv[:], dtv[:], 1e-4, 10.0, Alu.max, Alu.min)

            # flow normalization
            fl2 = small.tile([128, NB, 3], f32)
            nc.vector.tensor_tensor(fl2[:], flow_p, flow_p, Alu.mult)
            vn = small.tile([128, NB, 1], f32)
            nc.vector.tensor_reduce(vn[:], fl2[:], axis=AX.X, op=Alu.add)
            nc.scalar.activation(vn[:], vn[:], Act.Sqrt, bias=zero128[:], scale=1.0)
            nc.vector.tensor_scalar(vn[:], vn[:], 1e-6, None, Alu.max)
            rv = small.tile([128, NB, 1], f32)
            nc.vector.reciprocal(rv[:], vn[:])
            vhat = small.tile([128, NB, 3], f32)
            nc.vector.tensor_tensor(vhat[:], flow_p,
                                    rv[:].broadcast_to((128, NB, 3)), Alu.mult)

            # global advection gate
            pfg = ps(1, NB * 5)
            nc.tensor.matmul(pfg[:], ones128[:], hp[:].rearrange("p a b -> p (a b)"),
                             start=True, stop=True)
            fgrow = small.tile([1, NB, 5], f32)
            nc.vector.tensor_copy(fgrow[:], pfg[:])
            fgm_r = small.tile([1, 5], f32)
            nc.vector.tensor_reduce(
                fgm_r[:], fgrow[:].transpose([0, 2, 1]),
                axis=AX.X, op=Alu.add)
            nc.vector.tensor_scalar(fgm_r[:], fgm_r[:], 1.0 / N, None, Alu.mult)
            pft = ps(5, 1)
            nc.tensor.matmul(pft[:], fgm_r[0:1, :], ones128[0:1, 0:1],
                             is_transpose=True)
            fgm = small.tile([5, 1], f32)
            nc.vector.tensor_copy(fgm[:], pft[:])
            pg1 = ps(C, 1)
            nc.tensor.matmul(pg1[:], Wg1T[:], fgm[:], start=True, stop=True)
            hg = small.tile([C, 1], f32)
            nc.scalar.activation(hg[:], pg1[:], Act.Relu, bias=bg1_v[:], scale=1.0)
            pg2 = ps(C, 1)
            nc.tensor.matmul(pg2[:], Wg2T[:], hg[:], start=True, stop=True)
            fgf = small.tile([C, 1], f32)
            nc.vector.tensor_scalar(fgf[:], pg2[:], bg2_v[:], None, Alu.add)
            # TR = gate * fgf (on Act), transposed to point layout
            nc.scalar.activation(gateT[:], gateT[:], Act.Identity,
                                 bias=zero128[0:C, :], scale=fgf[:])
            TRp = big.tile([128, NB, C], f32)
            for j in range(3):
                pt = ps(128, 512)
                for q in range(8):
                    b = 8 * j + q
                    nc.tensor.matmul(pt[:, C * q:C * (q + 1)],
                                     gateT[:, 128 * b:128 * (b + 1)],
                                     ident[0:C, 0:C], is_transpose=True)
                nc.scalar.copy(TRp[:, 8 * j:8 * (j + 1), :], pt[:])

            # ---------------- S2 pass A + pipelined pass C ---------------
            s1stk.close()
            loopstk = contextlib.ExitStack()
            gpool = loopstk.enter_context(tc.tile_pool(name="gth", bufs=3))
            pc = loopstk.enter_context(tc.tile_pool(name="passc", bufs=1))
            ngsb = loopstk.enter_context(tc.tile_pool(name="ngsb", bufs=3))
            fnp = loopstk.enter_context(tc.tile_pool(name="fnp", bufs=2))
            idx_all = big.tile([128, NB * K], u16)
            # groups of blocks sharing one gather (last slice split for tail)
            groups = [(0, 4), (4, 4), (8, 4), (12, 4), (16, 4), (20, 2), (22, 2)]
            g_off = [0]
            for _, nb_ in groups:
                g_off.append(g_off[-1] + K * 8 * nb_)
            idx_dram = dram.tile([g_off[-1] * 16], i16)
            idx_wrap = big.tile([128, g_off[-1]], i16)
            agg = big.tile([128, NB, C], f32)
            dp = small.tile([128, NB, 2], f32)
            x_sb = latex.tile([C, N], f32)
            xs6 = small.tile([C, 6], f32)
            x2s6 = small.tile([C, 6], f32)
            gth_tiles = {}
            sl_data = {}
            _fa = []

            def stage_write(g, bl):
                # DRAM addr = base + S*(p%16) + 8*nb*k + 8*bl + p//16
                b0, nb = groups[g]
                S = K * 8 * nb
                base = g_off[g] * 16
                nc.sync.dma_start(
                    AP(idx_dram.tensor, base + 8 * bl,
                       [[1, 8], [S, 16], [8 * nb, K]]),
                    idx_all[:, (b0 + bl) * K:(b0 + bl + 1) * K].bitcast(i16))

            def stage_and_gather(g):
                b0, nb = groups[g]
                S = K * 8 * nb
                base = g_off[g] * 16
                for grp in range(8):
                    nc.sync.dma_start(
                        idx_wrap[16 * grp:16 * (grp + 1),
                                 g_off[g]:g_off[g + 1]].rearrange(
                            "p (a q) -> p a q", q=8 * nb),
                        AP(idx_dram.tensor, base,
                           [[S, 16], [8 * nb, K], [1, 8 * nb]]))
                gth_h = []
                for hh in range(2):
                    gthh = gpool.tile([128, 8 * nb * 128], f32, tag="gth")
                    gth_h.append(gthh)
                    nc.gpsimd.ap_gather(
                        gthh[:],
                        fxT[:],
                        idx_wrap[:, g_off[g] + S // 2 * hh:
                                 g_off[g] + S // 2 * (hh + 1)],
                        channels=128, num_elems=N, d=1,
                        num_idxs=8 * nb * 128)
                gth_tiles[g] = gth_h

            def part1(g, half=None):
                """PE transposes + Act copies into fp16/fp32 point-major."""
                b0, nb = groups[g]
                if half in (None, 0):
                    gth_h = gth_tiles[g]
                    fnei = fnp.tile([128, nb, C, K], f16, tag="fnei")
                    fnxyz = fnp.tile([128, nb, 3, K], f32, tag="fnxyz")
                    sl_data[g] = (fnei, fnxyz)
                else:
                    fnei, fnxyz = sl_data[g]
                    gth_h = gth_tiles.pop(g)
                kqr = (range(K // 2) if half is None else
                       range(4 * half, 4 * (half + 1)))
                for kq2 in kqr:
                    ptg2 = ps(128, 2 * nb * C)
                    ptgx = ps(128, 2 * nb * 3)
                    for k2 in range(2):
                        kq = 2 * kq2 + k2
                        gth = gth_h[kq // 8]
                        kqh = kq % 8
                        for q in range(nb):
                            chunk = gth[:, kqh * nb * 128 + 128 * q:
                                        kqh * nb * 128 + 128 * (q + 1)]
                            nc.tensor.matmul(
                                ptg2[:, (k2 * nb + q) * C:(k2 * nb + q + 1) * C],
                                chunk, ident[:, 0:C], is_transpose=True)
                            nc.tensor.matmul(
                                ptgx[:, (k2 * nb + q) * 3:(k2 * nb + q + 1) * 3],
                                chunk, ident[:, C:C + 3], is_transpose=True)
                    nc.scalar.copy(
                        fnei[:, :, :, 2 * kq2:2 * kq2 + 2],
                        ptg2[:].rearrange("p (k b c) -> p b c k", k=2, c=C))
                    nc.scalar.copy(
                        fnxyz[:, :, :, 2 * kq2:2 * kq2 + 2],
                        ptgx[:].rearrange("p (k b c) -> p b c k", k=2, c=3))

            def part2(g):
                """softmax weights + aggregation + dist stats for group g."""
                b0, nb = groups[g]
                fnei, fnxyz = sl_data.pop(g)
                vhat = _s1b["vhat"]
                de16 = _s1b["de16"]
                dxyz = pc.tile([128, nb, 3, K], f32, tag="dxyz")
                nc.vector.tensor_tensor(
                    dxyz[:], fnxyz[:],
                    xyz_sb[:, b0:b0 + nb, :].unsqueeze(3).broadcast_to(
                        (128, nb, 3, K)), Alu.subtract)
                t3 = pc.tile([128, nb, 3, K], f32, tag="t3")
                nc.vector.tensor_tensor(
                    t3[:], dxyz[:],
                    vhat[:, b0:b0 + nb, :].unsqueeze(3).broadcast_to(
                        (128, nb, 3, K)), Alu.mult)
                numv = pc.tile([128, nb, K], f32, tag="numv")
                nc.vector.tensor_tensor(numv[:], t3[:, :, 0, :],
                                        t3[:, :, 1, :], Alu.add)
                nc.vector.tensor_tensor(numv[:], numv[:], t3[:, :, 2, :],
                                        Alu.add)
                nc.vector.tensor_tensor(t3[:], dxyz[:], dxyz[:], Alu.mult)
                d2k = pc.tile([128, nb, K], f32, tag="d2k")
                nc.vector.tensor_tensor(d2k[:], t3[:, :, 0, :],
                                        t3[:, :, 1, :], Alu.add)
                nc.vector.tensor_tensor(d2k[:], d2k[:], t3[:, :, 2, :], Alu.add)
                rden = pc.tile([128, nb, K], f32, tag="rden")
                nc.vector.tensor_scalar(rden[:], d2k[:], 1e-12, None, Alu.max)
                nc.scalar.activation(rden[:], rden[:], Act.Ln,
                                     bias=zero128[:], scale=1.0)
                nc.scalar.activation(rden[:], rden[:], Act.Exp,
                                     bias=zero128[:], scale=-0.5)
                sqd = pc.tile([128, nb, K], f32, tag="sqd")
                nc.vector.tensor_tensor(sqd[:], d2k[:], rden[:], Alu.mult)
                ek = pc.tile([128, nb, K], f32, tag="ek")
                nc.vector.tensor_tensor(ek[:], numv[:], rden[:], Alu.mult)
                nc.scalar.activation(ek[:], ek[:], Act.Exp,
                                     bias=zero128[:], scale=1.0 / TAU)
                se = pc.tile([128, nb, 1], f32, tag="se")
                nc.vector.tensor_reduce(se[:], ek[:], axis=AX.X, op=Alu.add)
                rse = pc.tile([128, nb, 1], f32, tag="rse")
                nc.vector.reciprocal(rse[:], se[:])
                nc.vector.tensor_tensor(ek[:], ek[:],
                                        rse[:].broadcast_to((128, nb, K)),
                                        Alu.mult)
                uw16 = pc.tile([128, nb, K], f16, tag="uw16")
                nc.vector.tensor_tensor(
                    uw16[:], ek[:],
                    de16[:, b0:b0 + nb, :].broadcast_to((128, nb, K)), Alu.add)
                # weighted aggregation (fp16 2x tree)
                prod = pc.tile([128, nb, C, K], f16, tag="prod")
                nc.vector.tensor_tensor(
                    prod[:], fnei[:],
                    uw16[:].unsqueeze(2).broadcast_to((128, nb, C, K)),
                    Alu.mult)
                s1t = pc.tile([128, nb, C, 8], f16, tag="s1t")
                nc.vector.tensor_tensor(s1t[:], prod[:, :, :, 0:8],
                                        prod[:, :, :, 8:16], Alu.add)
                s2t = pc.tile([128, nb, C, 4], f16, tag="s2t")
                nc.vector.tensor_tensor(s2t[:], s1t[:, :, :, 0:4],
                                        s1t[:, :, :, 4:8], Alu.add)
                s3t = pc.tile([128, nb, C, 2], f16, tag="s3t")
                nc.vector.tensor_tensor(s3t[:], s2t[:, :, :, 0:2],
                                        s2t[:, :, :, 2:4], Alu.add)
                nc.vector.tensor_tensor(agg[:, b0:b0 + nb, :],
                                        s3t[:, :, :, 0], s3t[:, :, :, 1],
                                        Alu.add)
                if debug:
                    uwf = pc.tile([128, nb, K], f32, tag="uwf")
                    nc.vector.tensor_copy(uwf[:], uw16[:])
                    nc.sync.dma_start(
                        AP(dbg_ext["d_uw"], b0 * K,
                           [[NB * K, 128], [K, nb], [1, K]]), uwf[:])
                    nc.sync.dma_start(
                        AP(dbg_ext["d_numv"], b0 * K,
                           [[NB * K, 128], [K, nb], [1, K]]), numv[:])
                # dist stats
                ndsl = dp[:, b0:b0 + nb, 0:1]
                nvsl = dp[:, b0:b0 + nb, 1:2]
                nc.vector.tensor_reduce(ndsl, sqd[:], axis=AX.X, op=Alu.add)
                nc.vector.tensor_scalar(ndsl, ndsl, 1.0 / K, None, Alu.mult)
                d2m = pc.tile([128, nb, 1], f32, tag="d2m")
                nc.vector.tensor_reduce(d2m[:], d2k[:], axis=AX.X, op=Alu.add)
                nc.vector.tensor_scalar(d2m[:], d2m[:], 1.0 / K, None, Alu.mult)
                nd2 = pc.tile([128, nb, 1], f32, tag="nd2")
                nc.vector.tensor_tensor(nd2[:], ndsl, ndsl, Alu.mult)
                nc.vector.tensor_tensor(nvsl, d2m[:], nd2[:], Alu.subtract)

            def emit_conv(s):
                """dist transpose + reaction conv for column chunk s."""
                headsT = _s1b["headsT"]
                sl = slice(512 * s, 512 * (s + 1))
                ptd = ps(2, 512)
                for q in range(4):
                    nc.tensor.matmul(ptd[:, 128 * q:128 * (q + 1)],
                                     dp[:, 4 * s + q:4 * s + q + 1, :],
                                     ident[:, :], is_transpose=True)
                nc.scalar.copy(headsT[0:2, sl], ptd[:])
                px = ps(C, 512)
                nc.tensor.matmul(px[:], Wr1fT[:], fT[:, sl], start=True,
                                 stop=False)
                nc.tensor.matmul(px[:], Wr1hT[:], headsT[:, sl],
                                 start=False, stop=True)
                nc.scalar.activation(x_sb[:, sl], px[:], Act.Copy, bias=0.0,
                                     scale=1.0, accum_out=xs6[:, s:s + 1])
                sqscr = pc.tile([C, 512], f32, tag="sqscr")
                nc.scalar.activation(sqscr[:], x_sb[:, sl], Act.Square,
                                     bias=zero128[0:C, :], scale=1.0,
                                     accum_out=x2s6[:, s:s + 1])

            # triggers keyed by block index
            g_end = {groups[g][0] + groups[g][1] - 1: g for g in range(len(groups))}
            part1_at = {}
            part2_at = {}
            part1b_at = {}
            for g in range(len(groups)):
                e = groups[g][0] + groups[g][1] - 1
                d1 = 3 if groups[g][1] == 4 else 2
                if e + d1 < NB:
                    part1_at.setdefault(e + d1 - 1, []).append(g)
                    part1b_at.setdefault(e + d1, []).append(g)
                if e + d1 + 2 < NB:
                    part2_at.setdefault(e + d1 + 2, []).append(g)
                elif e + d1 + 1 < NB:
                    part2_at.setdefault(e + d1 + 1, []).append(g)

            for b in range(NB):
                if b == 4:
                    emit_s1b()
                if b == 8:
                    de1x = small.tile([128, NB, 1], f32)
                    nc.vector.tensor_scalar(de1x[:], _s1b["de"][:], 1.0, None,
                                            Alu.add)
                    alphax = small.tile([128, NB, 1], f32)
                    nc.vector.tensor_scalar(alphax[:], de1x[:], dtv[:, 0:1],
                                            None, Alu.mult)
                    nc.vector.tensor_scalar(alphax[:], alphax[:], -1.0, 1.0,
                                            Alu.mult, Alu.add)
                    fa = latex.tile([128, NB, C], f32)
                    nc.gpsimd.tensor_tensor(
                        fa[:], f_sb[:],
                        alphax[:].broadcast_to((128, NB, C)), Alu.mult)
                    nc.gpsimd.tensor_scalar(_s1b["TRp"][:], _s1b["TRp"][:],
                                            dtv[:, 0:1], None, Alu.mult)
                    nc.gpsimd.tensor_tensor(fa[:], fa[:], _s1b["TRp"][:],
                                            Alu.add)
                    _fa.append(fa)
                for g in part1_at.get(b, []):
                    part1(g, half=0)
                for g in part1b_at.get(b, []):
                    part1(g, half=1)
                cand = small.tile([128, 64], f32, tag="cand", bufs=2)
                segloc = small.tile([128, 64], u16, tag="segloc", bufs=2)
                for h in range(2):
                    ngp = ngps.tile([128, 1536], f32, tag="ngp")
                    for j in range(3):
                        cj = 3 * h + j
                        nc.tensor.matmul(ngp[:, 512 * j:512 * (j + 1)],
                                         A36[:, 128 * b:128 * (b + 1)],
                                         B36[:, 512 * cj:512 * (cj + 1)],
                                         start=True, stop=True)
                    negd = ngsb.tile([128, 1536], f32, tag="negd")
                    nc.scalar.copy(negd[:], ngp[:])
                    for s in range(4):
                        s8 = 4 * h + s
                        nc.vector.max(cand[:, 8 * s8:8 * (s8 + 1)],
                                      negd[:, 384 * s:384 * (s + 1)])
                        nc.vector.max_index(segloc[:, 8 * s8:8 * (s8 + 1)],
                                            cand[:, 8 * s8:8 * (s8 + 1)],
                                            negd[:, 384 * s:384 * (s + 1)])
                jc16 = small.tile([128, 64], u16, tag="jc16", bufs=2)
                nc.vector.tensor_tensor(jc16[:], segloc[:], segb64u[:], Alu.add)
                v16 = small.tile([128, 16], f32, tag="v16", bufs=2)
                mrc = small.tile([128, 64], f32, tag="mrc", bufs=2)
                cp16 = small.tile([128, 16], u16, tag="cp16", bufs=2)
                nc.vector.max(v16[:, 0:8], cand[:])
                nc.vector.max_index(cp16[:, 0:8], v16[:, 0:8], cand[:])
                nc.vector.match_replace(mrc[:], v16[:, 0:8], cand[:], -1e30)
                nc.vector.max(v16[:, 8:16], mrc[:])
                nc.vector.max_index(cp16[:, 8:16], v16[:, 8:16], mrc[:])
                rankmap = small.tile([128, 64], i16, tag="rankmap", bufs=2)
                nc.gpsimd.local_scatter(rankmap[:], rank16[:],
                                        cp16[:].bitcast(i16),
                                        channels=128, num_elems=64, num_idxs=16)
                nc.gpsimd.tensor_scalar(rankmap[:], rankmap[:], 1, None,
                                        Alu.subtract)
                nc.gpsimd.local_scatter(idx_all[:, K * b:K * (b + 1)].bitcast(i16),
                                        jc16[:].bitcast(i16), rankmap[:],
                                        channels=128, num_elems=16, num_idxs=64)
                gg = 0
                while groups[gg][0] + groups[gg][1] <= b:
                    gg += 1
                stage_write(gg, b - groups[gg][0])
                if b in g_end:
                    stage_and_gather(g_end[b])
                for g in part2_at.get(b, []):
                    part2(g)
                    if g <= 4:
                        emit_conv(g)

            for g in range(len(groups)):
                if g in gth_tiles or g in sl_data:
                    if g in gth_tiles and g not in sl_data:
                        part1(g)
                    elif g in gth_tiles:
                        part1(g, half=1)
                    part2(g)
                    if g <= 4:
                        emit_conv(g)
            emit_conv(5)
            headsT = _s1b["headsT"]
            de = _s1b["de"]
            TRp = _s1b["TRp"]

            # ---------------- S3 BN + reaction tail + combine ------------
            loopstk.close()
            late = ctx.enter_context(tc.tile_pool(name="late", bufs=2))
            fa = _fa[0]
            bn_loc = small.tile([C, 2], f32)
            nc.vector.tensor_reduce(bn_loc[:, 0:1], xs6[:], axis=AX.X, op=Alu.add)
            nc.vector.tensor_reduce(bn_loc[:, 1:2], x2s6[:], axis=AX.X, op=Alu.add)
            bn_in = dram.tile([C, 2], f32)
            bn_out = dram.tile([C, 2], f32)
            bn_g = small.tile([C, 2], f32)
            if nocol:
                nc.vector.tensor_scalar(bn_g[:], bn_loc[:], float(NCORES), None,
                                        Alu.mult)
            else:
                nc.sync.dma_start(bn_in[:], bn_loc[:])
                nc.gpsimd.collective_compute(
                    "AllReduce", Alu.add, replica_groups=[list(range(NCORES))],
                    ins=[bn_in[:].opt()], outs=[bn_out[:].opt()])
                nc.sync.dma_start(bn_g[:], bn_out[:])
            Mtot = float(NCORES * N)
            mu = small.tile([C, 1], f32)
            nc.vector.tensor_scalar(mu[:], bn_g[:, 0:1], 1.0 / Mtot, None, Alu.mult)
            var = small.tile([C, 1], f32)
            nc.vector.tensor_scalar(var[:], bn_g[:, 1:2], 1.0 / Mtot, None, Alu.mult)
            mu2 = small.tile([C, 1], f32)
            nc.vector.tensor_tensor(mu2[:], mu[:], mu[:], Alu.mult)
            nc.vector.tensor_tensor(var[:], var[:], mu2[:], Alu.subtract)
            nc.vector.tensor_scalar(var[:], var[:], BN_EPS, None, Alu.add)
            rstd = small.tile([C, 1], f32)
            nc.scalar.activation(rstd[:], var[:], Act.Ln,
                                 bias=zero128[0:C, :], scale=1.0)
            nc.scalar.activation(rstd[:], rstd[:], Act.Exp,
                                 bias=zero128[0:C, :], scale=-0.5)
            s_vec = small.tile([C, 1], f32)
            nc.vector.tensor_tensor(s_vec[:], gamma_v[:], rstd[:], Alu.mult)
            b_vec = small.tile([C, 1], f32)
            nc.vector.tensor_tensor(b_vec[:], mu[:], s_vec[:], Alu.mult)
            nc.vector.tensor_tensor(b_vec[:], beta_v[:], b_vec[:], Alu.subtract)

            # per-chunk: relu -> conv2 -> +br2 -> transpose -> combine -> out
            out_sb = late.tile([128, NB, C], f32)
            for j in range(6):
                sl = slice(512 * j, 512 * (j + 1))
                nc.scalar.activation(x_sb[:, sl], x_sb[:, sl], Act.Relu,
                                     bias=b_vec[:], scale=s_vec[:])
                pr = ps(C, 512)
                nc.tensor.matmul(pr[:], Wr2T[:], x_sb[:, sl], start=True, stop=True)
                scr = late.tile([C, 512], f32, tag="scr")
                nc.scalar.activation(scr[:], pr[:], Act.Identity,
                                     bias=br2_v[:], scale=1.0)
                pt = ps(128, 4 * C)
                for q in range(4):
                    nc.tensor.matmul(pt[:, C * q:C * (q + 1)],
                                     scr[:, 128 * q:128 * (q + 1)],
                                     ident[0:C, 0:C], is_transpose=True)
                nc.vector.tensor_copy(out_sb[:, 4 * j:4 * (j + 1), :], pt[:])
                bsl = slice(4 * j, 4 * (j + 1))
                nc.vector.tensor_tensor(agg[:, bsl, :], agg[:, bsl, :],
                                        out_sb[:, bsl, :], Alu.add)
                nc.vector.scalar_tensor_tensor(out_sb[:, bsl, :], agg[:, bsl, :],
                                               dtv[:], fa[:, bsl, :],
                                               Alu.mult, Alu.add)
                nc.sync.dma_start(
                    AP(out_ext, 512 * j * C, [[C, 128], [128 * C, 4], [1, C]]),
                    out_sb[:, bsl, :])

    nc.compile()
    return nc


@functools.cache
def _get_nc(debug=False):
    return _build(debug=debug)


def _run(nc, inputs, trace=False):
    from concourse.bass_utils import run_bass_kernel_spmd
    f_seq = np.ascontiguousarray(np.asarray(inputs["f_seq"], dtype=np.float32))
    xyz = np.ascontiguousarray(np.asarray(inputs["xyz"], dtype=np.float32))
    in_maps = []
    for core in range(NCORES):
        b, l = divmod(core, L)
        m = {"f": f_seq[b, l], "xyz": xyz[b, l]}
        for k in WEIGHT_NAMES:
            m[k] = np.ascontiguousarray(
                np.asarray(inputs[k], dtype=np.float32).reshape(-1))
        in_maps.append(m)
    return run_bass_kernel_spmd(nc, in_maps, core_ids=list(range(NCORES)),
                                trace=trace)


def kernel(**inputs):
    nc = _get_nc()
    res = _run(nc, inputs)
    out = np.stack([np.asarray(res.results[i]["out"]) for i in range(NCORES)])
    return out.reshape(B, L, N, C).astype(np.float32)
